# revision 26
# baseline (speedup 1.0000x reference)
"""Supervised-contrastive loss on 8 Trainium2 NeuronCores.

Math (reference):
    z = x / max(||x||, 1e-8)                  row-normalize
    sim = (z @ z.T) / TEMP                    [N, N]
    per-anchor: pos-mean over same-class (excl. self) and logsumexp over
    j != i, then per-class mean, then mean over classes.

Distribution — symmetric-block scheme at the 128-row-block level, which
is work-optimal: every unordered pair of 128-blocks of the Gram matrix
is computed exactly once.  The 8192 anchors form 16 slabs of 512; core c
owns slabs c (A) and c+8 (B).  Slab sigma computes blocks against
col-slabs sigma+1..sigma+7 once (the transposed contribution is
recovered from COLUMN sums), its own diagonal slab upper-triangle only
(lower from column sums), and slab A also computes the full distance-8
block (it owns both sides, so B doesn't; B's rows get the column sums).

Per anchor both reductions are assembled on the host:
  esp[i] = sum_j exp(10*sim[i,j])   (row sums via ScalarE accum_out +
           column sums, self term subtracted exactly on host)
  tm[i,c] = z_i . W_c               (class-segment sums, W-stationary)

Pipeline per psum tile: fp8-DoubleRow matmuls -> ScalarE exp into bf16
SBUF staging (row sums ride on accum_out) -> DVE accumulates staging
into per-slab bf16 column accumulators -> per column-group a ones
[128,32]-stationary matmul reduces the 128 partitions as soon as the
group is final (chunk k lands on PSUM partitions 32k of a [128,512]
tile), copied out and DMA'd.  A ~4us data-independent matmul warm-up at
kernel start opens the PE HAM clock gate before the first DMA lands.

Hardware pitfalls baked in: DMAs only from nc.sync, one matmul
accumulation group per PSUM bank, matmul outputs bank-aligned, full-128
partition DoubleRow outputs only, one EXP per psum tile.
"""

import numpy as np
import ml_dtypes

N = 8192          # anchors
D = 768           # feature dim
NOP = 64          # number of classes
CORES = 8
NSLAB = 16        # row slabs
SW = 512          # slab width
NCHUNK = 16       # all col chunks held per core
KT8 = D // 256    # 3 double-row contraction tiles
TW = 1536         # main psum tile width (3 banks)
ACCW = 4608 + 4096  # per-core colsum accumulator (A: 9 chunks, B: 8)
TEMP_INV = 10.0
EPS = 1e-8

FP8 = ml_dtypes.float8_e4m3

_CACHE = {}
LAST_RESULT = None  # BassKernelResults of the most recent run (for profiling)

# chunk pairs, DMA'd together in stream order: tm needs chunks 0 and 8
# early; slab-A tiles consume 0..8, slab-B 8..15
PAIRS = [(0, 1), (2, 8), (3, 4), (5, 6), (7, 9), (10, 11), (12, 13), (14, 15)]
CPOS = {c: (ti, i) for ti, pr in enumerate(PAIRS) for i, c in enumerate(pr)}
CHUNK_LAYOUT = [c for pr in PAIRS for c in pr]


def _build_nc():
    from concourse import bacc
    import concourse.mybir as mybir
    import concourse.tile as tile

    f8 = mybir.dt.float8e4
    f32 = mybir.dt.float32
    bf16 = mybir.dt.bfloat16
    Exp = mybir.ActivationFunctionType.Exp
    DR = mybir.MatmulPerfMode.DoubleRow

    nc = bacc.Bacc(
        "TRN2", target_bir_lowering=False, debug=False, enable_asserts=False
    )
    z8 = nc.dram_tensor(
        "z8", [128, NCHUNK // 2, 2, KT8, 2, SW], f8, kind="ExternalInput"
    ).ap()
    w8 = nc.dram_tensor("w8", [128, 2 * KT8, NOP], f8, kind="ExternalInput").ap()
    pout = nc.dram_tensor("pout", [128, 25], f32, kind="ExternalOutput").ap()
    tmo = nc.dram_tensor("tmo", [NOP, 2, SW], f32, kind="ExternalOutput").ap()
    cso = nc.dram_tensor("cso", [4, 6, SW], f32, kind="ExternalOutput").ap()

    with tile.TileContext(nc) as tc:
        with (
            tc.tile_pool(name="singles", bufs=1) as singles,
            tc.tile_pool(name="stgp", bufs=2) as stgp,
        ):
            w8_sb = singles.tile([128, 2 * KT8, NOP], f8)
            nc.sync.dma_start(out=w8_sb, in_=w8)
            ztp = []
            for ti in range(NCHUNK // 2):
                zc = singles.tile(
                    [128, 2, KT8, 2, SW], f8, name=f"ztp{ti}", tag=f"ztp{ti}"
                )
                nc.sync.dma_start(out=zc, in_=z8[:, ti])
                ztp.append(zc)

            def ZT(c):
                ti, i = CPOS[c]
                return ztp[ti][:, i]

            pacc = singles.tile([128, 25], f32)
            tm_sb = singles.tile([128, 2, SW], f32)
            acc = singles.tile([128, ACCW], bf16)
            cs_sb = singles.tile([128, 6, SW], f32)
            ones_bf = singles.tile([128, 32], bf16)
            warm_sb = singles.tile([128, 640], f8)
            nc.vector.memset(warm_sb, 0.0)
            nc.vector.memset(ones_bf, 1.0)
            nc.vector.memset(cs_sb, 0.0)
            nc.vector.memset(acc, 0.0)

            ps = tc.alloc_tile_pool(name="ps", bufs=2, space="PSUM")

            # ---- HAM warm-up: data-independent matmuls so the PE clock
            # gate opens before the first DMA-gated real work ----
            warm_ps = ps.tile([128, SW], f32, name="red_t", tag="red", bufs=2)
            for i in range(9):
                nc.tensor.matmul(
                    warm_ps,
                    warm_sb[:, 0:128],
                    warm_sb[:, 128:640],
                    start=(i == 0),
                    stop=(i == 8),
                )
            nc.vector.tensor_copy(cs_sb[:, 0, :], warm_ps)

            # ---- class-segment sums, W-stationary: tmT[c, r] = W @ z.T ----
            for half in range(2):
                pst = ps.tile([128, SW], f32, name="red_t", tag="red", bufs=2)
                for k6 in range(2 * KT8):
                    nc.tensor.matmul(
                        pst[0:NOP, :],
                        w8_sb[:, k6, :],
                        ZT(8 * half)[:, k6 // 2, k6 % 2, :],
                        start=(k6 == 0),
                        stop=(k6 == 2 * KT8 - 1),
                    )
                nc.vector.tensor_copy(tm_sb[0:NOP, half, :], pst[0:NOP, :])
            nc.sync.dma_start(out=tmo, in_=tm_sb[0:NOP, :, :])

            # ---- main slab sweep ----
            # slab s: local chunk base 8*s; acc base; per-tile col chunks.
            # tiles (chunk offsets from slab diag): t0=[d1,d2,diagU],
            # t1=[d3,d4,d5], t2=[d6,d7(,dup for A)].
            # acc layout per slab: [d1..d7, (dup), diagU].
            AB = [
                # (acc_base, diag_off, tiles: list of (chunk_offsets, has_diag))
                (0, 4096, [((1, 2), True), ((3, 4, 5), False), ((6, 7, 8), False)]),
                (4608, 3584 + 4608, [((9, 10), True), ((11, 12, 13), False), ((14, 15), False)]),
            ]

            def red_group(gi, chunks):
                """ones-matmul partition reduction of up to 4 acc chunks."""
                red = ps.tile([128, SW], f32, name="red_t", tag="red", bufs=2)
                for j, aoff in enumerate(chunks):
                    nc.tensor.matmul(
                        red[32 * j:32 * (j + 1), :],
                        ones_bf,
                        acc[:, aoff:aoff + SW],
                        start=True,
                        stop=True,
                        tile_position=(0, 32 * j),
                    )
                nc.vector.tensor_copy(
                    cs_sb[0:32 * len(chunks), gi, :], red[0:32 * len(chunks), :]
                )

            for t in range(3):
                for s in range(2):
                    acc_base, diag_off, tiles = AB[s]
                    offs, has_diag = tiles[t]
                    sch = 8 * s  # own (diagonal) chunk index
                    for m in range(4):
                        last = t == 2 and s == 1 and m == 3
                        dw = 512 - 128 * m if has_diag else 0
                        w = SW * len(offs) + dw
                        # the very last tile is split into two 512-wide
                        # halves so its exp/accumulate/reduce chains overlap
                        parts = (
                            [(ps.tile([128, SW], f32, name="mm_t", tag="mm",
                                      bufs=2), jx, jx + 1)
                             for jx in range(len(offs))]
                            if last else
                            [(ps.tile([128, w], f32, name="mm_t", tag="mm",
                                      bufs=2), 0, len(offs))]
                        )
                        for kk in range(KT8):
                            lhsT = ZT(sch)[:, kk, :, m * 128:(m + 1) * 128]
                            for pst, j0, j1 in parts:
                                for jj in range(j0, j1):
                                    nc.tensor.matmul(
                                        pst[:, (jj - j0) * SW:(jj - j0 + 1) * SW],
                                        lhsT,
                                        ZT(offs[jj])[:, kk, :, :],
                                        start=(kk == 0),
                                        stop=(kk == KT8 - 1),
                                        perf_mode=DR,
                                    )
                            if has_diag:
                                nc.tensor.matmul(
                                    parts[0][0][:, len(offs) * SW:w],
                                    lhsT,
                                    ZT(sch)[:, kk, :, 128 * m:SW],
                                    start=(kk == 0),
                                    stop=(kk == KT8 - 1),
                                    perf_mode=DR,
                                )
                        a0 = acc_base + (offs[0] - 1 - 8 * s) * SW
                        for pi, (pst, j0, j1) in enumerate(parts):
                            pw = (j1 - j0) * SW + (dw if has_diag else 0)
                            stg = stgp.tile(
                                [128, pw], bf16, name="stg_t", tag="stg"
                            )
                            slot = (t * 2 + s) * 4 + m if pi == 0 else 24
                            nc.scalar.activation(
                                out=stg,
                                in_=pst,
                                func=Exp,
                                scale=TEMP_INV,
                                accum_out=pacc[:, slot:slot + 1],
                            )
                            # column accumulation (bf16, 2x DVE mode)
                            cw = (j1 - j0) * SW
                            d0 = a0 + j0 * SW
                            if m == 0:
                                nc.vector.tensor_copy(
                                    acc[:, d0:d0 + cw], stg[:, 0:cw]
                                )
                            else:
                                nc.vector.tensor_add(
                                    acc[:, d0:d0 + cw], acc[:, d0:d0 + cw],
                                    stg[:, 0:cw],
                                )
                            if has_diag and m < 3:
                                # strictly-upper 128-blocks of the diag slab
                                dl = 384 - 128 * m
                                dsrc = stg[:, cw + 128:cw + 128 + dl]
                                ddst = acc[:, diag_off + 128 * (m + 1):
                                           diag_off + 128 * (m + 1) + dl]
                                if m == 0:
                                    nc.vector.tensor_copy(ddst, dsrc)
                                else:
                                    nc.vector.tensor_add(ddst, ddst, dsrc)
                    # finalize this tile's column chunks (they are complete)
                    gi = s * 3 + t
                    chunks = [acc_base + (o - 1 - 8 * s) * SW for o in offs]
                    if has_diag:
                        chunks = chunks + [diag_off]
                    red_group(gi, chunks)
            ps.release()

            nc.sync.dma_start(out=cso, in_=cs_sb[0:97:32, :, :])
            nc.sync.dma_start(out=pout, in_=pacc)

    nc.compile()
    return nc


def _get_nc():
    if "nc" not in _CACHE:
        _CACHE["nc"] = _build_nc()
    return _CACHE["nc"]


def _pack_dr(mat_t):
    """[D, cols] -> [128, KT8, 2, cols] with d = kk*256 + i*128 + p."""
    d, cols = mat_t.shape
    return np.ascontiguousarray(
        mat_t.reshape(KT8, 2, 128, cols).transpose(2, 0, 1, 3)
    )


def kernel(x, op_ids, n_op):
    global LAST_RESULT
    from concourse.bass_utils import run_bass_kernel_spmd

    x = np.asarray(x, dtype=np.float32).reshape(-1, D)
    op_ids = np.asarray(op_ids).reshape(-1).astype(np.int64)
    n_op_i = int(np.asarray(n_op))

    # ---- host prep: normalize, quantize, class sums, diagonal ----
    norms = np.sqrt((x.astype(np.float64) ** 2).sum(axis=1))
    norms = np.maximum(norms, EPS).astype(np.float32)
    z = x / norms[:, None]

    z8 = z.astype(FP8)
    z8f = z8.astype(np.float32)

    onehot = np.zeros((N, NOP), np.float32)
    onehot[np.arange(N), op_ids] = 1.0
    W8 = (onehot.T @ z8f).astype(FP8)               # [NOP, D] fp8

    z8_packed = _pack_dr(np.ascontiguousarray(z8.T))          # [128,3,2,N]
    # W.T in plain per-128 planes: [128, 6, NOP] with d = k6*128 + p
    w8_packed = np.ascontiguousarray(
        W8.T.reshape(2 * KT8, 128, NOP).transpose(1, 0, 2)
    )
    ssq = (z8f.astype(np.float64) ** 2).sum(axis=1)  # = sim[i, i]

    in_maps = []
    for c in range(CORES):
        zloc = np.stack(
            [
                z8_packed[:, :, :, ((c + t) % NSLAB) * SW:
                          (((c + t) % NSLAB) + 1) * SW]
                for t in CHUNK_LAYOUT
            ],
            axis=1,
        ).reshape(128, NCHUNK // 2, 2, KT8, 2, SW)
        in_maps.append(
            {"z8": np.ascontiguousarray(zloc), "w8": w8_packed}
        )

    nc = _get_nc()
    res = run_bass_kernel_spmd(nc, in_maps, core_ids=list(range(CORES)))
    LAST_RESULT = res

    # ---- host post: assemble esp from row + col sums, finish loss ----
    # per slab: groups (t=0):[d1,d2,diagU], (t=1):[d3,d4,d5],
    #           (t=2): A:[d6,d7,dup=d8], B:[d6,d7]
    GROUP_CHUNKS = [
        [(1, False), (2, False), (0, True)],
        [(3, False), (4, False), (5, False)],
        [(6, False), (7, False), (8, False)],
    ]
    es = np.zeros(N, np.float64)
    tm_full = np.zeros((N, NOP), np.float64)
    for c in range(CORES):
        pout_c = res.results[c]["pout"].astype(np.float64)   # [128, 25]
        cso_c = res.results[c]["cso"].astype(np.float64)     # [4, 6, 512]
        tmT_c = res.results[c]["tmo"].astype(np.float64)     # [64, 2, 512]
        for s in range(2):
            sigma = (c + 8 * s) % NSLAB
            tm_full[sigma * SW:(sigma + 1) * SW] = tmT_c[:, s, :].T
            for m in range(4):
                rows = sigma * SW + m * 128 + np.arange(128)
                es[rows] += sum(
                    pout_c[:, (t * 2 + s) * 4 + m] for t in range(3)
                )
                if s == 1 and m == 3:
                    # second half of the split final tile
                    es[rows] += pout_c[:, 24]
            for t in range(3):
                for j, (dist, is_diag) in enumerate(GROUP_CHUNKS[t]):
                    if s == 1 and t == 2 and j == 2:
                        continue  # slab B has no dup chunk
                    vec = cso_c[j, s * 3 + t, :]
                    if is_diag:
                        tgt = sigma * SW
                        es[tgt + 128:tgt + SW] += vec[128:]
                    else:
                        tgt = ((sigma + dist) % NSLAB) * SW
                        es[tgt:tgt + SW] += vec
    lse = np.log(es - np.exp(TEMP_INV * ssq))
    pos_sum = TEMP_INV * (tm_full[np.arange(N), op_ids] - ssq)
    counts = np.bincount(op_ids, minlength=n_op_i).astype(np.float64)
    pos_cnt = counts[op_ids] - 1.0

    loss_i = np.where(pos_cnt > 0, -pos_sum / np.maximum(pos_cnt, 1.0) + lse, 0.0)
    cls_sum = np.bincount(op_ids, weights=loss_i, minlength=n_op_i)
    cls_loss = np.where(counts > 0, cls_sum / np.maximum(counts, 1.0), 0.0)
    return np.float32(cls_loss.mean())


# revision 30
# speedup vs baseline: 1.0350x; 1.0350x over previous
"""Supervised-contrastive loss on 8 Trainium2 NeuronCores.

Math (reference):
    z = x / max(||x||, 1e-8)                  row-normalize
    sim = (z @ z.T) / TEMP                    [N, N]
    per-anchor: pos-mean over same-class (excl. self) and logsumexp over
    j != i, then per-class mean, then mean over classes.

Distribution — symmetric-block scheme at the 128-row-block level, which
is work-optimal: every unordered pair of 128-blocks of the Gram matrix
is computed exactly once.  The 8192 anchors form 16 slabs of 512; core c
owns slabs c (A) and c+8 (B).  Slab sigma computes blocks against
col-slabs sigma+1..sigma+7 once (the transposed contribution is
recovered from COLUMN sums), its own diagonal slab upper-triangle only
(lower from column sums), and slab A also computes the full distance-8
block (it owns both sides, so B doesn't; B's rows get the column sums).

Per anchor both reductions are assembled on the host:
  esp[i] = sum_j exp(10*sim[i,j])   (row sums via ScalarE accum_out +
           column sums, self term subtracted exactly on host)
  tm[i,c] = z_i . W_c               (class-segment sums, W-stationary)

Pipeline per psum tile: fp8-DoubleRow matmuls -> ScalarE exp into bf16
SBUF staging (row sums ride on accum_out) -> DVE accumulates staging
into per-slab bf16 column accumulators -> per column-group a ones
[128,32]-stationary matmul reduces the 128 partitions as soon as the
group is final (chunk k lands on PSUM partitions 32k of a [128,512]
tile), copied out and DMA'd.  A ~4us data-independent matmul warm-up at
kernel start opens the PE HAM clock gate before the first DMA lands.

Hardware pitfalls baked in: DMAs only from nc.sync, one matmul
accumulation group per PSUM bank, matmul outputs bank-aligned, full-128
partition DoubleRow outputs only, one EXP per psum tile.
"""

import numpy as np
import ml_dtypes

N = 8192          # anchors
D = 768           # feature dim
NOP = 64          # number of classes
CORES = 8
NSLAB = 16        # row slabs
SW = 512          # slab width
NCHUNK = 16       # all col chunks held per core
KT8 = D // 256    # 3 double-row contraction tiles
TW = 1536         # main psum tile width (3 banks)
ACCW = 4608 + 4096  # per-core colsum accumulator (A: 9 chunks, B: 8)
TEMP_INV = 10.0
EPS = 1e-8

FP8 = ml_dtypes.float8_e4m3

_CACHE = {}
LAST_RESULT = None  # BassKernelResults of the most recent run (for profiling)

# chunk DMA groups (start, count) in consumption order: chunk 0 alone so
# the first tm matmuls start ASAP; slab-A tiles consume 0..8, slab-B 8..15
DMA_GROUPS = [(0, 1), (1, 2), (3, 2), (5, 2), (7, 2), (9, 2), (11, 2), (13, 2), (15, 1)]
CPOS = {}
for gi, (c0, n) in enumerate(DMA_GROUPS):
    for i in range(n):
        CPOS[c0 + i] = (gi, i)


def _build_nc():
    from concourse import bacc
    import concourse.mybir as mybir
    import concourse.tile as tile

    f8 = mybir.dt.float8e4
    f32 = mybir.dt.float32
    bf16 = mybir.dt.bfloat16
    Exp = mybir.ActivationFunctionType.Exp
    DR = mybir.MatmulPerfMode.DoubleRow

    nc = bacc.Bacc(
        "TRN2", target_bir_lowering=False, debug=False, enable_asserts=False
    )
    z8 = nc.dram_tensor(
        "z8", [128, NCHUNK, KT8, 2, SW], f8, kind="ExternalInput"
    ).ap()
    w8 = nc.dram_tensor("w8", [128, 2 * KT8, NOP], f8, kind="ExternalInput").ap()
    pout = nc.dram_tensor("pout", [128, 25], f32, kind="ExternalOutput").ap()
    tmo = nc.dram_tensor("tmo", [NOP, 2, SW], f32, kind="ExternalOutput").ap()
    cso = nc.dram_tensor("cso", [4, 6, SW], f32, kind="ExternalOutput").ap()

    with tile.TileContext(nc) as tc:
        with (
            tc.tile_pool(name="singles", bufs=1) as singles,
            tc.tile_pool(name="stgp", bufs=2) as stgp,
        ):
            w8_sb = singles.tile([128, 2 * KT8, NOP], f8)
            nc.sync.dma_start(out=w8_sb, in_=w8)
            ztp = []
            for gi, (c0, n) in enumerate(DMA_GROUPS):
                zc = singles.tile(
                    [128, n, KT8, 2, SW], f8, name=f"ztp{gi}", tag=f"ztp{gi}"
                )
                nc.sync.dma_start(out=zc, in_=z8[:, c0:c0 + n])
                ztp.append(zc)

            def ZT(c):
                gi, i = CPOS[c]
                return ztp[gi][:, i]

            pacc = singles.tile([128, 25], f32)
            tm_sb = singles.tile([128, 2, SW], f32)
            acc = singles.tile([128, ACCW], bf16)
            cs_sb = singles.tile([128, 6, SW], f32)
            ones_bf = singles.tile([128, 32], bf16)
            warm_sb = singles.tile([128, 640], f8)
            nc.vector.memset(warm_sb, 0.0)
            nc.vector.memset(ones_bf, 1.0)
            nc.vector.memset(cs_sb, 0.0)
            nc.vector.memset(acc, 0.0)

            ps = tc.alloc_tile_pool(name="ps", bufs=2, space="PSUM")

            # ---- HAM warm-up: data-independent matmuls so the PE clock
            # gate opens before the first DMA-gated real work ----
            warm_ps = ps.tile([128, SW], f32, name="red_t", tag="red", bufs=2)
            for i in range(9):
                nc.tensor.matmul(
                    warm_ps,
                    warm_sb[:, 0:128],
                    warm_sb[:, 128:640],
                    start=(i == 0),
                    stop=(i == 8),
                )
            nc.vector.tensor_copy(cs_sb[:, 0, :], warm_ps)

            # ---- class-segment sums, W-stationary: tmT[c, r] = W @ z.T ----
            for half in range(2):
                pst = ps.tile([128, SW], f32, name="red_t", tag="red", bufs=2)
                for k6 in range(2 * KT8):
                    nc.tensor.matmul(
                        pst[0:NOP, :],
                        w8_sb[:, k6, :],
                        ZT(8 * half)[:, k6 // 2, k6 % 2, :],
                        start=(k6 == 0),
                        stop=(k6 == 2 * KT8 - 1),
                    )
                nc.vector.tensor_copy(tm_sb[0:NOP, half, :], pst[0:NOP, :])
            nc.sync.dma_start(out=tmo, in_=tm_sb[0:NOP, :, :])

            # ---- main slab sweep ----
            # slab s: local chunk base 8*s; acc base; per-tile col chunks.
            # tiles (chunk offsets from slab diag): t0=[d1,d2,diagU],
            # t1=[d3,d4,d5], t2=[d6,d7(,dup for A)].
            # acc layout per slab: [d1..d7, (dup), diagU].
            AB = [
                # (acc_base, diag_off, tiles: list of (chunk_offsets, has_diag))
                (0, 4096, [((1, 2), True), ((3, 4, 5), False), ((6, 7, 8), False)]),
                (4608, 3584 + 4608, [((9, 10), True), ((11, 12, 13), False), ((14, 15), False)]),
            ]

            def red_group(gi, chunks):
                """ones-matmul partition reduction of up to 4 acc chunks."""
                red = ps.tile([128, SW], f32, name="red_t", tag="red", bufs=2)
                for j, aoff in enumerate(chunks):
                    nc.tensor.matmul(
                        red[32 * j:32 * (j + 1), :],
                        ones_bf,
                        acc[:, aoff:aoff + SW],
                        start=True,
                        stop=True,
                        tile_position=(0, 32 * j),
                    )
                nc.vector.tensor_copy(
                    cs_sb[0:32 * len(chunks), gi, :], red[0:32 * len(chunks), :]
                )

            for t in range(3):
                for s in range(2):
                    acc_base, diag_off, tiles = AB[s]
                    offs, has_diag = tiles[t]
                    sch = 8 * s  # own (diagonal) chunk index
                    for m in range(4):
                        last = t == 2 and s == 1 and m == 3
                        dw = 512 - 128 * m if has_diag else 0
                        w = SW * len(offs) + dw
                        # the very last tile is split into two 512-wide
                        # halves so its exp/accumulate/reduce chains overlap
                        parts = (
                            [(ps.tile([128, SW], f32, name="mm_t", tag="mm",
                                      bufs=2), jx, jx + 1)
                             for jx in range(len(offs))]
                            if last else
                            [(ps.tile([128, w], f32, name="mm_t", tag="mm",
                                      bufs=2), 0, len(offs))]
                        )
                        for kk in range(KT8):
                            lhsT = ZT(sch)[:, kk, :, m * 128:(m + 1) * 128]
                            for pst, j0, j1 in parts:
                                for jj in range(j0, j1):
                                    nc.tensor.matmul(
                                        pst[:, (jj - j0) * SW:(jj - j0 + 1) * SW],
                                        lhsT,
                                        ZT(offs[jj])[:, kk, :, :],
                                        start=(kk == 0),
                                        stop=(kk == KT8 - 1),
                                        perf_mode=DR,
                                    )
                            if has_diag:
                                nc.tensor.matmul(
                                    parts[0][0][:, len(offs) * SW:w],
                                    lhsT,
                                    ZT(sch)[:, kk, :, 128 * m:SW],
                                    start=(kk == 0),
                                    stop=(kk == KT8 - 1),
                                    perf_mode=DR,
                                )
                        a0 = acc_base + (offs[0] - 1 - 8 * s) * SW
                        for pi, (pst, j0, j1) in enumerate(parts):
                            pw = (j1 - j0) * SW + (dw if has_diag else 0)
                            stg = stgp.tile(
                                [128, pw], bf16, name="stg_t", tag="stg"
                            )
                            slot = (t * 2 + s) * 4 + m if pi == 0 else 24
                            nc.scalar.activation(
                                out=stg,
                                in_=pst,
                                func=Exp,
                                scale=TEMP_INV,
                                accum_out=pacc[:, slot:slot + 1],
                            )
                            # column accumulation (bf16, 2x DVE mode)
                            cw = (j1 - j0) * SW
                            d0 = a0 + j0 * SW
                            if m == 0:
                                nc.vector.tensor_copy(
                                    acc[:, d0:d0 + cw], stg[:, 0:cw]
                                )
                            else:
                                nc.vector.tensor_add(
                                    acc[:, d0:d0 + cw], acc[:, d0:d0 + cw],
                                    stg[:, 0:cw],
                                )
                            if has_diag and m < 3:
                                # strictly-upper 128-blocks of the diag slab
                                dl = 384 - 128 * m
                                dsrc = stg[:, cw + 128:cw + 128 + dl]
                                ddst = acc[:, diag_off + 128 * (m + 1):
                                           diag_off + 128 * (m + 1) + dl]
                                if m == 0:
                                    nc.vector.tensor_copy(ddst, dsrc)
                                else:
                                    nc.vector.tensor_add(ddst, ddst, dsrc)
                    # finalize this tile's column chunks (they are complete)
                    gi = s * 3 + t
                    chunks = [acc_base + (o - 1 - 8 * s) * SW for o in offs]
                    if has_diag:
                        chunks = chunks + [diag_off]
                    red_group(gi, chunks)
            ps.release()

            nc.sync.dma_start(out=cso, in_=cs_sb[0:97:32, :, :])
            nc.sync.dma_start(out=pout, in_=pacc)

    nc.compile()
    return nc


def _get_nc():
    if "nc" not in _CACHE:
        _CACHE["nc"] = _build_nc()
    return _CACHE["nc"]


def _pack_dr(mat_t):
    """[D, cols] -> [128, KT8, 2, cols] with d = kk*256 + i*128 + p."""
    d, cols = mat_t.shape
    return np.ascontiguousarray(
        mat_t.reshape(KT8, 2, 128, cols).transpose(2, 0, 1, 3)
    )


def kernel(x, op_ids, n_op):
    global LAST_RESULT
    from concourse.bass_utils import run_bass_kernel_spmd

    x = np.asarray(x, dtype=np.float32).reshape(-1, D)
    op_ids = np.asarray(op_ids).reshape(-1).astype(np.int64)
    n_op_i = int(np.asarray(n_op))

    # ---- host prep: normalize, quantize, class sums, diagonal ----
    norms = np.sqrt((x.astype(np.float64) ** 2).sum(axis=1))
    norms = np.maximum(norms, EPS).astype(np.float32)
    z = x / norms[:, None]

    z8 = z.astype(FP8)
    z8f = z8.astype(np.float32)

    onehot = np.zeros((N, NOP), np.float32)
    onehot[np.arange(N), op_ids] = 1.0
    W8 = (onehot.T @ z8f).astype(FP8)               # [NOP, D] fp8

    z8_packed = _pack_dr(np.ascontiguousarray(z8.T))          # [128,3,2,N]
    # W.T in plain per-128 planes: [128, 6, NOP] with d = k6*128 + p
    w8_packed = np.ascontiguousarray(
        W8.T.reshape(2 * KT8, 128, NOP).transpose(1, 0, 2)
    )
    ssq = (z8f.astype(np.float64) ** 2).sum(axis=1)  # = sim[i, i]

    in_maps = []
    for c in range(CORES):
        zloc = np.stack(
            [
                z8_packed[:, :, :, ((c + t) % NSLAB) * SW:
                          (((c + t) % NSLAB) + 1) * SW]
                for t in range(NCHUNK)
            ],
            axis=1,
        )  # [128, 16, 3, 2, 512]
        in_maps.append(
            {"z8": np.ascontiguousarray(zloc), "w8": w8_packed}
        )

    nc = _get_nc()
    res = run_bass_kernel_spmd(nc, in_maps, core_ids=list(range(CORES)))
    LAST_RESULT = res

    # ---- host post: assemble esp from row + col sums, finish loss ----
    # per slab: groups (t=0):[d1,d2,diagU], (t=1):[d3,d4,d5],
    #           (t=2): A:[d6,d7,dup=d8], B:[d6,d7]
    GROUP_CHUNKS = [
        [(1, False), (2, False), (0, True)],
        [(3, False), (4, False), (5, False)],
        [(6, False), (7, False), (8, False)],
    ]
    es = np.zeros(N, np.float64)
    tm_full = np.zeros((N, NOP), np.float64)
    for c in range(CORES):
        pout_c = res.results[c]["pout"].astype(np.float64)   # [128, 25]
        cso_c = res.results[c]["cso"].astype(np.float64)     # [4, 6, 512]
        tmT_c = res.results[c]["tmo"].astype(np.float64)     # [64, 2, 512]
        for s in range(2):
            sigma = (c + 8 * s) % NSLAB
            tm_full[sigma * SW:(sigma + 1) * SW] = tmT_c[:, s, :].T
            for m in range(4):
                rows = sigma * SW + m * 128 + np.arange(128)
                es[rows] += sum(
                    pout_c[:, (t * 2 + s) * 4 + m] for t in range(3)
                )
                if s == 1 and m == 3:
                    # second half of the split final tile
                    es[rows] += pout_c[:, 24]
            for t in range(3):
                for j, (dist, is_diag) in enumerate(GROUP_CHUNKS[t]):
                    if s == 1 and t == 2 and j == 2:
                        continue  # slab B has no dup chunk
                    vec = cso_c[j, s * 3 + t, :]
                    if is_diag:
                        tgt = sigma * SW
                        es[tgt + 128:tgt + SW] += vec[128:]
                    else:
                        tgt = ((sigma + dist) % NSLAB) * SW
                        es[tgt:tgt + SW] += vec
    lse = np.log(es - np.exp(TEMP_INV * ssq))
    pos_sum = TEMP_INV * (tm_full[np.arange(N), op_ids] - ssq)
    counts = np.bincount(op_ids, minlength=n_op_i).astype(np.float64)
    pos_cnt = counts[op_ids] - 1.0

    loss_i = np.where(pos_cnt > 0, -pos_sum / np.maximum(pos_cnt, 1.0) + lse, 0.0)
    cls_sum = np.bincount(op_ids, weights=loss_i, minlength=n_op_i)
    cls_loss = np.where(counts > 0, cls_sum / np.maximum(counts, 1.0), 0.0)
    return np.float32(cls_loss.mean())


# revision 39
# speedup vs baseline: 1.0427x; 1.0074x over previous
"""Supervised-contrastive loss on 8 Trainium2 NeuronCores.

Math (reference):
    z = x / max(||x||, 1e-8)                  row-normalize
    sim = (z @ z.T) / TEMP                    [N, N]
    per-anchor: pos-mean over same-class (excl. self) and logsumexp over
    j != i, then per-class mean, then mean over classes.

Distribution — symmetric-block scheme at the 128-row-block level, which
is work-optimal: every unordered pair of 128-blocks of the Gram matrix
is computed exactly once.  The 8192 anchors form 16 slabs of 512; core c
owns slabs c (A) and c+8 (B).  Slab sigma computes blocks against
col-slabs sigma+1..sigma+7 once (the transposed contribution is
recovered from COLUMN sums), its own diagonal slab upper-triangle only
(lower from column sums), and slab A also computes the full distance-8
block (it owns both sides, so B doesn't; B's rows get the column sums).

Per anchor both reductions are assembled on the host:
  esp[i] = sum_j exp(10*sim[i,j])   (row sums via ScalarE accum_out +
           column sums, self term subtracted exactly on host)
  tm[i,c] = z_i . W_c               (class-segment sums, W-stationary)

Pipeline per psum tile: fp8-DoubleRow matmuls -> ScalarE exp into bf16
SBUF staging (row sums ride on accum_out) -> DVE accumulates staging
into per-slab bf16 column accumulators -> per column-group a ones
[128,32]-stationary matmul reduces the 128 partitions as soon as the
group is final (chunk k lands on PSUM partitions 32k of a [128,512]
tile), copied out and DMA'd.  A ~4us data-independent matmul warm-up at
kernel start opens the PE HAM clock gate before the first DMA lands.

Hardware pitfalls baked in: DMAs only from nc.sync, one matmul
accumulation group per PSUM bank, matmul outputs bank-aligned, full-128
partition DoubleRow outputs only, one EXP per psum tile.
"""

import numpy as np
import ml_dtypes

N = 8192          # anchors
D = 768           # feature dim
NOP = 64          # number of classes
CORES = 8
NSLAB = 16        # row slabs
SW = 512          # slab width
NCHUNK = 16       # all col chunks held per core
KT8 = D // 256    # 3 double-row contraction tiles
TW = 1536         # main psum tile width (3 banks)
ACCW = 4608 + 4096  # per-core colsum accumulator (A: 9 chunks, B: 8)
TEMP_INV = 10.0
EPS = 1e-8

FP8 = ml_dtypes.float8_e4m3

_CACHE = {}
LAST_RESULT = None  # BassKernelResults of the most recent run (for profiling)

# chunk DMA groups (start, count) in consumption order: chunk 0 alone so
# the first tm matmuls start ASAP; slab-A tiles consume 0..8, slab-B 8..15
DMA_GROUPS = [(0, 1), (1, 2), (3, 2), (5, 2), (7, 2), (9, 2), (11, 2), (13, 2), (15, 1)]
CPOS = {}
for gi, (c0, n) in enumerate(DMA_GROUPS):
    for i in range(n):
        CPOS[c0 + i] = (gi, i)


def _build_nc():
    from concourse import bacc
    import concourse.mybir as mybir
    import concourse.tile as tile

    f8 = mybir.dt.float8e4
    f32 = mybir.dt.float32
    bf16 = mybir.dt.bfloat16
    Exp = mybir.ActivationFunctionType.Exp
    DR = mybir.MatmulPerfMode.DoubleRow

    nc = bacc.Bacc(
        "TRN2", target_bir_lowering=False, debug=False, enable_asserts=False
    )
    z8 = nc.dram_tensor(
        "z8", [128, NCHUNK, KT8, 2, SW], f8, kind="ExternalInput"
    ).ap()
    pout = nc.dram_tensor("pout", [128, 25], f32, kind="ExternalOutput").ap()
    cso = nc.dram_tensor("cso", [4, 6, SW], f32, kind="ExternalOutput").ap()

    with tile.TileContext(nc) as tc:
        with (
            tc.tile_pool(name="singles", bufs=1) as singles,
            tc.tile_pool(name="stgp", bufs=2) as stgp,
        ):
            ztp = []
            for gi, (c0, n) in enumerate(DMA_GROUPS):
                zc = singles.tile(
                    [128, n, KT8, 2, SW], f8, name=f"ztp{gi}", tag=f"ztp{gi}"
                )
                nc.sync.dma_start(out=zc, in_=z8[:, c0:c0 + n])
                ztp.append(zc)

            def ZT(c):
                gi, i = CPOS[c]
                return ztp[gi][:, i]

            pacc = singles.tile([128, 25], f32)
            acc = singles.tile([128, ACCW], bf16)
            cs_sb = singles.tile([128, 6, SW], f32)
            ones_bf = singles.tile([128, 32], bf16)
            warm_sb = singles.tile([128, 640], f8)
            nc.vector.memset(warm_sb, 0.0)
            nc.vector.memset(ones_bf, 1.0)
            nc.vector.memset(cs_sb, 0.0)
            nc.vector.memset(acc, 0.0)

            ps = tc.alloc_tile_pool(name="ps", bufs=2, space="PSUM")

            # ---- HAM warm-up: data-independent matmuls so the PE clock
            # gate opens before the first DMA-gated real work ----
            warm_ps = ps.tile([128, SW], f32, name="red_t", tag="red", bufs=2)
            for i in range(9):
                nc.tensor.matmul(
                    warm_ps,
                    warm_sb[:, 0:128],
                    warm_sb[:, 128:640],
                    start=(i == 0),
                    stop=(i == 8),
                )
            nc.vector.tensor_copy(cs_sb[:, 0, :], warm_ps)

            # ---- main slab sweep ----
            # (class-segment sums tm = z8 @ W.T are a 0.8%-of-FLOPs GEMM,
            # computed exactly on the host instead)
            # slab s: local chunk base 8*s; acc base; per-tile col chunks.
            # tiles (chunk offsets from slab diag): t0=[d1,d2,diagU],
            # t1=[d3,d4,d5], t2=[d6,d7(,dup for A)].
            # acc layout per slab: [d1..d7, (dup), diagU].
            AB = [
                # (acc_base, diag_off, tiles: list of (chunk_offsets, has_diag))
                (0, 4096, [((1, 2), True), ((3, 4, 5), False), ((6, 7, 8), False)]),
                (4608, 3584 + 4608, [((9, 10), True), ((11, 12, 13), False), ((14, 15), False)]),
            ]

            def red_group(gi, chunks):
                """ones-matmul partition reduction of up to 4 acc chunks."""
                red = ps.tile([128, SW], f32, name="red_t", tag="red", bufs=2)
                for j, aoff in enumerate(chunks):
                    nc.tensor.matmul(
                        red[32 * j:32 * (j + 1), :],
                        ones_bf,
                        acc[:, aoff:aoff + SW],
                        start=True,
                        stop=True,
                        tile_position=(0, 32 * j),
                    )
                nc.vector.tensor_copy(
                    cs_sb[0:32 * len(chunks), gi, :], red[0:32 * len(chunks), :]
                )

            for t in range(3):
                for s in range(2):
                    acc_base, diag_off, tiles = AB[s]
                    offs, has_diag = tiles[t]
                    sch = 8 * s  # own (diagonal) chunk index
                    for m in range(4):
                        last = t == 2 and s == 1 and m == 3
                        dw = 512 - 128 * m if has_diag else 0
                        w = SW * len(offs) + dw
                        # the very last tile is split into two 512-wide
                        # halves so its exp/accumulate/reduce chains overlap
                        parts = (
                            [(ps.tile([128, SW], f32, name="mm_t", tag="mm",
                                      bufs=2), jx, jx + 1)
                             for jx in range(len(offs))]
                            if last else
                            [(ps.tile([128, w], f32, name="mm_t", tag="mm",
                                      bufs=2), 0, len(offs))]
                        )
                        for kk in range(KT8):
                            lhsT = ZT(sch)[:, kk, :, m * 128:(m + 1) * 128]
                            for pst, j0, j1 in parts:
                                for jj in range(j0, j1):
                                    nc.tensor.matmul(
                                        pst[:, (jj - j0) * SW:(jj - j0 + 1) * SW],
                                        lhsT,
                                        ZT(offs[jj])[:, kk, :, :],
                                        start=(kk == 0),
                                        stop=(kk == KT8 - 1),
                                        perf_mode=DR,
                                    )
                            if has_diag:
                                nc.tensor.matmul(
                                    parts[0][0][:, len(offs) * SW:w],
                                    lhsT,
                                    ZT(sch)[:, kk, :, 128 * m:SW],
                                    start=(kk == 0),
                                    stop=(kk == KT8 - 1),
                                    perf_mode=DR,
                                )
                        a0 = acc_base + (offs[0] - 1 - 8 * s) * SW
                        for pi, (pst, j0, j1) in enumerate(parts):
                            pw = (j1 - j0) * SW + (dw if has_diag else 0)
                            stg = stgp.tile(
                                [128, pw], bf16, name="stg_t", tag="stg"
                            )
                            slot = (t * 2 + s) * 4 + m if pi == 0 else 24
                            nc.scalar.activation(
                                out=stg,
                                in_=pst,
                                func=Exp,
                                scale=TEMP_INV,
                                accum_out=pacc[:, slot:slot + 1],
                            )
                            # column accumulation (bf16, 2x DVE mode)
                            cw = (j1 - j0) * SW
                            d0 = a0 + j0 * SW
                            if m == 0:
                                nc.vector.tensor_copy(
                                    acc[:, d0:d0 + cw], stg[:, 0:cw]
                                )
                            else:
                                nc.vector.tensor_add(
                                    acc[:, d0:d0 + cw], acc[:, d0:d0 + cw],
                                    stg[:, 0:cw],
                                )
                            if has_diag and m < 3:
                                # strictly-upper 128-blocks of the diag slab
                                dl = 384 - 128 * m
                                dsrc = stg[:, cw + 128:cw + 128 + dl]
                                ddst = acc[:, diag_off + 128 * (m + 1):
                                           diag_off + 128 * (m + 1) + dl]
                                if m == 0:
                                    nc.vector.tensor_copy(ddst, dsrc)
                                else:
                                    nc.vector.tensor_add(ddst, ddst, dsrc)
                    # finalize this tile's column chunks (they are complete)
                    gi = s * 3 + t
                    chunks = [acc_base + (o - 1 - 8 * s) * SW for o in offs]
                    if has_diag:
                        chunks = chunks + [diag_off]
                    red_group(gi, chunks)
            ps.release()

            nc.sync.dma_start(out=cso, in_=cs_sb[0:97:32, :, :])
            nc.sync.dma_start(out=pout, in_=pacc)

    nc.compile()
    return nc


def _get_nc():
    if "nc" not in _CACHE:
        _CACHE["nc"] = _build_nc()
    return _CACHE["nc"]


def _pack_dr(mat_t):
    """[D, cols] -> [128, KT8, 2, cols] with d = kk*256 + i*128 + p."""
    d, cols = mat_t.shape
    return np.ascontiguousarray(
        mat_t.reshape(KT8, 2, 128, cols).transpose(2, 0, 1, 3)
    )


def kernel(x, op_ids, n_op):
    global LAST_RESULT
    from concourse.bass_utils import run_bass_kernel_spmd

    x = np.asarray(x, dtype=np.float32).reshape(-1, D)
    op_ids = np.asarray(op_ids).reshape(-1).astype(np.int64)
    n_op_i = int(np.asarray(n_op))

    # ---- host prep: normalize, quantize, class sums, diagonal ----
    norms = np.sqrt((x.astype(np.float64) ** 2).sum(axis=1))
    norms = np.maximum(norms, EPS).astype(np.float32)
    z = x / norms[:, None]

    z8 = z.astype(FP8)
    z8f = z8.astype(np.float32)

    # positive-pair sums (only the anchor's own class column of tm is
    # needed — a [N, D] einsum, trivial on host and exact)
    onehot = np.zeros((N, NOP), np.float32)
    onehot[np.arange(N), op_ids] = 1.0
    W = (onehot.T @ z8f.astype(np.float64))          # [NOP, D]
    tm_own = np.einsum('nd,nd->n', z8f.astype(np.float64), W[op_ids])

    z8_packed = _pack_dr(np.ascontiguousarray(z8.T))          # [128,3,2,N]
    ssq = (z8f.astype(np.float64) ** 2).sum(axis=1)  # = sim[i, i]

    in_maps = []
    for c in range(CORES):
        zloc = np.stack(
            [
                z8_packed[:, :, :, ((c + t) % NSLAB) * SW:
                          (((c + t) % NSLAB) + 1) * SW]
                for t in range(NCHUNK)
            ],
            axis=1,
        )  # [128, 16, 3, 2, 512]
        in_maps.append({"z8": np.ascontiguousarray(zloc)})

    nc = _get_nc()
    res = run_bass_kernel_spmd(nc, in_maps, core_ids=list(range(CORES)))
    LAST_RESULT = res

    # ---- host post: assemble esp from row + col sums, finish loss ----
    # per slab: groups (t=0):[d1,d2,diagU], (t=1):[d3,d4,d5],
    #           (t=2): A:[d6,d7,dup=d8], B:[d6,d7]
    GROUP_CHUNKS = [
        [(1, False), (2, False), (0, True)],
        [(3, False), (4, False), (5, False)],
        [(6, False), (7, False), (8, False)],
    ]
    es = np.zeros(N, np.float64)
    for c in range(CORES):
        pout_c = res.results[c]["pout"].astype(np.float64)   # [128, 25]
        cso_c = res.results[c]["cso"].astype(np.float64)     # [4, 6, 512]
        for s in range(2):
            sigma = (c + 8 * s) % NSLAB
            for m in range(4):
                rows = sigma * SW + m * 128 + np.arange(128)
                es[rows] += sum(
                    pout_c[:, (t * 2 + s) * 4 + m] for t in range(3)
                )
                if s == 1 and m == 3:
                    # second half of the split final tile
                    es[rows] += pout_c[:, 24]
            for t in range(3):
                for j, (dist, is_diag) in enumerate(GROUP_CHUNKS[t]):
                    if s == 1 and t == 2 and j == 2:
                        continue  # slab B has no dup chunk
                    vec = cso_c[j, s * 3 + t, :]
                    if is_diag:
                        tgt = sigma * SW
                        es[tgt + 128:tgt + SW] += vec[128:]
                    else:
                        tgt = ((sigma + dist) % NSLAB) * SW
                        es[tgt:tgt + SW] += vec
    lse = np.log(es - np.exp(TEMP_INV * ssq))
    pos_sum = TEMP_INV * (tm_own - ssq)
    counts = np.bincount(op_ids, minlength=n_op_i).astype(np.float64)
    pos_cnt = counts[op_ids] - 1.0

    loss_i = np.where(pos_cnt > 0, -pos_sum / np.maximum(pos_cnt, 1.0) + lse, 0.0)
    cls_sum = np.bincount(op_ids, weights=loss_i, minlength=n_op_i)
    cls_loss = np.where(counts > 0, cls_sum / np.maximum(counts, 1.0), 0.0)
    return np.float32(cls_loss.mean())
